# revision 36
# baseline (speedup 1.0000x reference)
"""Trainium2 Bass kernel for nn_AdditiveCouplingLayer.

y = x; y[:, 1::2] += MLP(x[:, 0::2])  with a 512->1024->1024->512 relu MLP.

Strategy: data-parallel over 8 NeuronCores (batch 65536 -> 8192/core),
weights replicated. The MLP's first two layers run in "transposed
activation" space (features on partitions, batch on the free dim) so
every matmul uses the natural weight layout; the host supplies the
masked half of x pre-transposed and pre-cast to fp16. Layer 3 swaps the
matmul operand roles (h2 slice stationary, W3 moving) so the
translation comes out in natural [batch, feature] layout — no output
transpose needed. Matmuls run in fp16 (1 cycle/row on the PE vs 4 for
fp32) with fp32 PSUM accumulation; weights are pre-cast to fp16 on the
host. MODE "f16x3" upgrades to near-fp32 precision via a 3-term hi/lo
split (3x the matmul work).
"""

import os
import sys

sys.path.insert(0, "/opt/trn_rl_repo")

import numpy as np

B, D, F, H = 65536, 1024, 512, 1024
NCORES = 8
BPC = B // NCORES  # rows per core
TB = 512  # batch tile (matmul free dim)
NBT = BPC // TB  # batch tiles per core
MODE = os.environ.get("BASS_COUPLING_MODE", "f16")

_cache = {}


def _build(mode):
    import concourse.bacc as bacc
    import concourse.tile as tile
    import concourse.mybir as mybir

    dt = mybir.dt
    AF = mybir.ActivationFunctionType
    split = mode == "f16x3"

    nc = bacc.Bacc(
        "TRN2", target_bir_lowering=False, debug=False, num_devices=NCORES
    )

    x_d = nc.dram_tensor("x", [BPC, D], dt.float32, kind="ExternalInput").ap()
    mT_d = nc.dram_tensor("mT", [F, BPC], dt.float16, kind="ExternalInput").ap()
    if split:
        mTl_d = nc.dram_tensor("mTl", [F, BPC], dt.float16, kind="ExternalInput").ap()
    w_d = {}
    for name, shape in (("w1", [F, H]), ("w2", [H, H]), ("w3", [H, F])):
        w_d[name] = nc.dram_tensor(name, shape, dt.float16, kind="ExternalInput").ap()
        if split:
            w_d[name + "l"] = nc.dram_tensor(
                name + "l", shape, dt.float16, kind="ExternalInput"
            ).ap()
    b1_d = nc.dram_tensor("b1m", [128, H // 128], dt.float32, kind="ExternalInput").ap()
    b2_d = nc.dram_tensor("b2m", [128, H // 128], dt.float32, kind="ExternalInput").ap()
    b3r_d = nc.dram_tensor("b3rep", [128, F], dt.float32, kind="ExternalInput").ap()
    y_d = nc.dram_tensor("y", [BPC, D], dt.float32, kind="ExternalOutput").ap()

    with tile.TileContext(nc) as tc:
        with (
            tc.tile_pool(name="wpool", bufs=1) as wpool,
            tc.tile_pool(name="xpool", bufs=2) as xpool,
            tc.tile_pool(name="mpool", bufs=3 if mode == "f16" else 2) as mpool,
            tc.tile_pool(name="hpool", bufs=2) as hpool,
            tc.tile_pool(name="pmm", bufs=4, space="PSUM") as pmm,
        ):
            # --- resident weights/biases, all on the sync HWDGE queue.
            # Only W1 + biases load before the first tile's inputs; W2/W3
            # DMAs are deferred into the first loop iterations (issued
            # right after that tile's mT loads) so layer 1 of tile 0
            # starts after ~1.8MB of DMA instead of ~4.5MB. ---
            deferred_w = []

            def load_w(name, rows, cols, eng):
                """One big tile + ONE DMA per weight matrix (k-chunks land
                side by side in the free dim); returns per-k column slices.
                eng=None defers the issue (pushed onto deferred_w; drained
                from the scalar engine between the first L1 evictions)."""
                nk = rows // 128
                big = wpool.tile(
                    [128, nk * cols], dt.float16, tag=name, name=name
                )

                def issue(eng):
                    eng.dma_start(
                        big[:].rearrange("p (k c) -> p k c", k=nk),
                        w_d[name].rearrange("(k p) c -> p k c", p=128),
                    )

                if eng is None:
                    deferred_w.append(issue)
                else:
                    issue(eng)
                return [big[:, k * cols : (k + 1) * cols] for k in range(nk)]

            def load_b(name, ap, n):
                # host pre-transposes biases to [128, n/128] so this DMA is
                # contiguous (a "(m p) -> p m" rearrange here is a 4-byte-
                # element gather that takes ~10us and stalls the DMA ring)
                t = wpool.tile([128, n // 128], dt.float32, tag=name)
                nc.scalar.dma_start(t[:], ap[:])
                return t

            # PE warmup: junk matmuls on a zeroed scratch tile keep the PE
            # busy through its HAM activity window while the first real
            # DMAs are in flight, so real matmuls start at 2.4GHz.
            scratch = wpool.tile([128, TB], dt.float16, tag="scratch")
            nc.gpsimd.memset(scratch[:], 0.0)
            pwarm = pmm.tile([128, TB], dt.float32, tag="warm", bufs=1)
            for _ in range(16):
                nc.tensor.matmul(
                    pwarm[:], scratch[:, :128], scratch[:], start=True, stop=True
                )

            # Startup DMA order is the critical path: W1 chunks interleave
            # with the tile-0 mT loads on the sync queue (first real matmul
            # needs only w1[0]+mT[0]); W2/W3 issue from the scalar queue
            # but only AFTER the first L1 evictions (see post_evict), so
            # they don't steal HBM bandwidth from the W1/mT0 stream.
            w1t = load_w("w1", F, H, nc.sync)
            b1t = load_b("b1t", b1_d, H)
            b2t = load_b("b2t", b2_d, H)
            b3rep = wpool.tile([128, F], dt.float32, tag="b3rep")
            nc.scalar.dma_start(b3rep[:], b3r_d[:])
            w2t = load_w("w2", H, H, None)
            w3t = load_w("w3", H, F, None)
            if split:
                w1l = load_w("w1l", F, H, nc.sync)
                w2l = load_w("w2l", H, H, None)
                w3l = load_w("w3l", H, F, None)

            def mm_group(psum, pairs):
                n = len(pairs)
                for i, (lhsT, rhs) in enumerate(pairs):
                    nc.tensor.matmul(
                        psum[:], lhsT, rhs, start=(i == 0), stop=(i == n - 1)
                    )

            def layer(wt, wl, ins, ins_lo, bt, nout, oname, drain_deferred=False):
                """Transposed-space layer: out[m][feat128, TB] = relu(W.T@in + b)."""
                outs = []
                outs_lo = []
                nk = len(ins)
                for m in range(nout // 128):
                    p = pmm.tile([128, TB], dt.float32, tag="mm")
                    ms = slice(m * 128, (m + 1) * 128)
                    pairs = [(wt[k][:, ms], ins[k][:]) for k in range(nk)]
                    if split:
                        pairs += [(wt[k][:, ms], ins_lo[k][:]) for k in range(nk)]
                        pairs += [(wl[k][:, ms], ins[k][:]) for k in range(nk)]
                    mm_group(p, pairs)
                    o = hpool.tile([128, TB], dt.float16, tag=f"{oname}_{m}")
                    nc.scalar.activation(o[:], p[:], AF.Relu, bias=bt[:, m : m + 1])
                    if drain_deferred and deferred_w:
                        deferred_w.pop(0)(nc.scalar)
                    outs.append(o)
                    if split:
                        of = hpool.tile(
                            [128, TB], dt.float32, tag="hf_tmp", bufs=3
                        )
                        nc.scalar.activation(
                            of[:], p[:], AF.Relu, bias=bt[:, m : m + 1]
                        )
                        ol = hpool.tile([128, TB], dt.float16, tag=f"{oname}l_{m}")
                        nc.vector.tensor_sub(ol[:], of[:], o[:])
                        outs_lo.append(ol)
                return outs, outs_lo

            def l1_tile(bt_i):
                """mT loads + layer 1 for one batch tile (issued one tile
                ahead of layers 2/3 so the PE never stalls on the W2/W3
                arrival at startup, and mT is naturally prefetched)."""
                r0 = bt_i * TB
                mT = []
                mTl = []
                for j in range(4):
                    t = mpool.tile(
                        [128, TB], dt.float16, tag=f"m{j}", name=f"m{j}"
                    )
                    nc.sync.dma_start(
                        t[:], mT_d[j * 128 : (j + 1) * 128, r0 : r0 + TB]
                    )
                    mT.append(t)
                    if split:
                        tl = mpool.tile(
                            [128, TB], dt.float16, tag=f"ml{j}", name=f"ml{j}"
                        )
                        nc.sync.dma_start(
                            tl[:], mTl_d[j * 128 : (j + 1) * 128, r0 : r0 + TB]
                        )
                        mTl.append(tl)
                return layer(
                    w1t, w1l if split else None, mT, mTl, b1t, H, "h1",
                    drain_deferred=(bt_i == 0),
                )

            h1, h1l = l1_tile(0)
            for bt_i in range(NBT):
                r0 = bt_i * TB

                h1_next = l1_tile(bt_i + 1) if bt_i + 1 < NBT else None

                # x tile (natural layout, needed only for the residual
                # assembly — issued after the mT loads on the same queue)
                xb = []
                for i in range(4):
                    t = xpool.tile([128, D], dt.float32, tag=f"x{i}")
                    nc.sync.dma_start(
                        t[:], x_d[r0 + i * 128 : r0 + (i + 1) * 128, :]
                    )
                    xb.append(t)
                h2, h2l = layer(
                    w2t, w2l if split else None, h1, h1l, b2t, H, "h2"
                )

                # y is assembled IN PLACE in the x tiles (even columns are
                # already x): odd cols += b3, then += translation.
                for i in range(4):
                    nc.vector.tensor_add(
                        xb[i][:, 1:D:2], xb[i][:, 1:D:2], b3rep[:]
                    )

                # layer 3 in natural layout: stationary = h2 batch-slice,
                # moving = W3 tile  ->  psum[batch128, F]
                for i in range(4):
                    p = pmm.tile([128, F], dt.float32, tag="mm")
                    bs = slice(i * 128, (i + 1) * 128)
                    pairs = [(h2[k][:, bs], w3t[k][:]) for k in range(8)]
                    if split:
                        pairs += [(h2l[k][:, bs], w3t[k][:]) for k in range(8)]
                        pairs += [(h2[k][:, bs], w3l[k][:]) for k in range(8)]
                    mm_group(p, pairs)
                    nc.vector.tensor_add(xb[i][:, 1:D:2], xb[i][:, 1:D:2], p[:])
                    nc.sync.dma_start(
                        y_d[r0 + i * 128 : r0 + (i + 1) * 128, :], xb[i][:]
                    )

                if h1_next is not None:
                    h1, h1l = h1_next

    nc.compile()
    return nc


def _get(mode):
    if mode not in _cache:
        _cache[mode] = _build(mode)
    return _cache[mode]


def _in_maps(x, W1, b1, W2, b2, W3, b3):
    split = MODE == "f16x3"

    def prep_w(w):
        hi = np.asarray(w, dtype=np.float32).astype(np.float16)
        if not split:
            return {"": hi}
        lo = (np.asarray(w, dtype=np.float32) - hi.astype(np.float32)).astype(
            np.float16
        )
        return {"": hi, "l": lo}

    ws = {}
    for name, w in (("w1", W1), ("w2", W2), ("w3", W3)):
        for suf, arr in prep_w(w).items():
            ws[name + suf] = arr

    common = dict(
        ws,
        b1m=np.ascontiguousarray(np.asarray(b1, np.float32).reshape(-1, 128).T),
        b2m=np.ascontiguousarray(np.asarray(b2, np.float32).reshape(-1, 128).T),
        b3rep=np.ascontiguousarray(
            np.broadcast_to(np.asarray(b3, np.float32), (128, F))
        ),
    )
    x = np.ascontiguousarray(np.asarray(x, np.float32))
    in_maps = []
    for c in range(NCORES):
        xs = x[c * BPC : (c + 1) * BPC]
        masked_t = np.ascontiguousarray(xs[:, 0::2].T)  # [F, BPC] f32
        m = dict(common, x=xs, mT=masked_t.astype(np.float16))
        if split:
            m["mTl"] = (masked_t - m["mT"].astype(np.float32)).astype(np.float16)
        in_maps.append(m)
    return in_maps


def kernel(x, W1, b1, W2, b2, W3, b3):
    from concourse.bass_utils import run_bass_kernel_spmd

    nc = _get(MODE)
    res = run_bass_kernel_spmd(
        nc, _in_maps(x, W1, b1, W2, b2, W3, b3), core_ids=list(range(NCORES))
    )
    return np.concatenate([res.results[c]["y"] for c in range(NCORES)], axis=0)


# revision 38
# speedup vs baseline: 2.4598x; 2.4598x over previous
"""Trainium2 Bass kernel for nn_AdditiveCouplingLayer.

y = x; y[:, 1::2] += MLP(x[:, 0::2])  with a 512->1024->1024->512 relu MLP.

Strategy: data-parallel over 8 NeuronCores (batch 65536 -> 8192/core),
weights replicated. The MLP's first two layers run in "transposed
activation" space (features on partitions, batch on the free dim) so
every matmul uses the natural weight layout; the host supplies the
masked half of x pre-transposed and pre-cast to fp16. Layer 3 swaps the
matmul operand roles (h2 slice stationary, W3 moving) so the
translation comes out in natural [batch, feature] layout — no output
transpose needed. Matmuls run in fp16 (1 cycle/row on the PE vs 4 for
fp32) with fp32 PSUM accumulation; weights are pre-cast to fp16 on the
host. MODE "f16x3" upgrades to near-fp32 precision via a 3-term hi/lo
split (3x the matmul work).
"""

import os
import sys

sys.path.insert(0, "/opt/trn_rl_repo")

import numpy as np

B, D, F, H = 65536, 1024, 512, 1024
NCORES = 8
BPC = B // NCORES  # rows per core
TB = 512  # batch tile (matmul free dim)
NBT = BPC // TB  # batch tiles per core
MODE = os.environ.get("BASS_COUPLING_MODE", "f16")

_cache = {}


def _build(mode):
    import concourse.bacc as bacc
    import concourse.tile as tile
    import concourse.mybir as mybir

    dt = mybir.dt
    AF = mybir.ActivationFunctionType
    split = mode == "f16x3"

    nc = bacc.Bacc(
        "TRN2", target_bir_lowering=False, debug=False, num_devices=NCORES
    )

    x_d = nc.dram_tensor("x", [BPC, D], dt.float32, kind="ExternalInput").ap()
    mT_d = nc.dram_tensor("mT", [F, BPC], dt.float16, kind="ExternalInput").ap()
    if split:
        mTl_d = nc.dram_tensor("mTl", [F, BPC], dt.float16, kind="ExternalInput").ap()
    w_d = {}
    for name, shape in (("w1", [F, H]), ("w2", [H, H]), ("w3", [H, F])):
        w_d[name] = nc.dram_tensor(name, shape, dt.float16, kind="ExternalInput").ap()
        if split:
            w_d[name + "l"] = nc.dram_tensor(
                name + "l", shape, dt.float16, kind="ExternalInput"
            ).ap()
    b1_d = nc.dram_tensor("b1m", [128, H // 128], dt.float32, kind="ExternalInput").ap()
    b2_d = nc.dram_tensor("b2m", [128, H // 128], dt.float32, kind="ExternalInput").ap()
    b3r_d = nc.dram_tensor("b3rep", [128, F], dt.float32, kind="ExternalInput").ap()
    y_d = nc.dram_tensor("y", [BPC, D], dt.float32, kind="ExternalOutput").ap()

    with tile.TileContext(nc) as tc:
        with (
            tc.tile_pool(name="wpool", bufs=1) as wpool,
            tc.tile_pool(name="xpool", bufs=2) as xpool,
            tc.tile_pool(name="mpool", bufs=3 if mode == "f16" else 2) as mpool,
            tc.tile_pool(name="hpool", bufs=2) as hpool,
            tc.tile_pool(name="pmm", bufs=4, space="PSUM") as pmm,
        ):
            # --- resident weights/biases, all on the sync HWDGE queue.
            # Only W1 + biases load before the first tile's inputs; W2/W3
            # DMAs are deferred into the first loop iterations (issued
            # right after that tile's mT loads) so layer 1 of tile 0
            # starts after ~1.8MB of DMA instead of ~4.5MB. ---
            deferred_w = []

            def load_w(name, rows, cols, eng):
                """One big tile + ONE DMA per weight matrix (k-chunks land
                side by side in the free dim); returns per-k column slices.
                eng=None defers the issue (pushed onto deferred_w; drained
                from the scalar engine between the first L1 evictions)."""
                nk = rows // 128
                big = wpool.tile(
                    [128, nk * cols], dt.float16, tag=name, name=name
                )

                def issue(eng):
                    eng.dma_start(
                        big[:].rearrange("p (k c) -> p k c", k=nk),
                        w_d[name].rearrange("(k p) c -> p k c", p=128),
                    )

                if eng is None:
                    deferred_w.append(issue)
                else:
                    issue(eng)
                return [big[:, k * cols : (k + 1) * cols] for k in range(nk)]

            def load_b(name, ap, n):
                # host pre-transposes biases to [128, n/128] so this DMA is
                # contiguous (a "(m p) -> p m" rearrange here is a 4-byte-
                # element gather that takes ~10us and stalls the DMA ring)
                t = wpool.tile([128, n // 128], dt.float32, tag=name)
                nc.scalar.dma_start(t[:], ap[:])
                return t

            # PE warmup: junk matmuls on a zeroed scratch tile keep the PE
            # busy through its HAM activity window while the first real
            # DMAs are in flight, so real matmuls start at 2.4GHz.
            scratch = wpool.tile([128, TB], dt.float16, tag="scratch")
            nc.gpsimd.memset(scratch[:], 0.0)
            pwarm = pmm.tile([128, TB], dt.float32, tag="warm", bufs=1)
            for _ in range(16):
                nc.tensor.matmul(
                    pwarm[:], scratch[:, :128], scratch[:], start=True, stop=True
                )

            # Startup DMA order is the critical path: W1 chunks interleave
            # with the tile-0 mT loads on the sync queue (first real matmul
            # needs only w1[0]+mT[0]); W2/W3 issue from the scalar queue
            # but only AFTER the first L1 evictions (see post_evict), so
            # they don't steal HBM bandwidth from the W1/mT0 stream.
            w1t = load_w("w1", F, H, nc.sync)
            b1t = load_b("b1t", b1_d, H)
            b2t = load_b("b2t", b2_d, H)
            b3rep = wpool.tile([128, F], dt.float32, tag="b3rep")
            nc.scalar.dma_start(b3rep[:], b3r_d[:])
            w2t = load_w("w2", H, H, None)
            w3t = load_w("w3", H, F, None)
            if split:
                w1l = load_w("w1l", F, H, nc.sync)
                w2l = load_w("w2l", H, H, None)
                w3l = load_w("w3l", H, F, None)

            def mm_group(psum, pairs):
                n = len(pairs)
                for i, (lhsT, rhs) in enumerate(pairs):
                    nc.tensor.matmul(
                        psum[:], lhsT, rhs, start=(i == 0), stop=(i == n - 1)
                    )

            def layer(wt, wl, ins, ins_lo, bt, nout, oname, drain_deferred=False):
                """Transposed-space layer: out[m][feat128, TB] = relu(W.T@in + b)."""
                outs = []
                outs_lo = []
                nk = len(ins)
                for m in range(nout // 128):
                    p = pmm.tile([128, TB], dt.float32, tag="mm")
                    ms = slice(m * 128, (m + 1) * 128)
                    pairs = [(wt[k][:, ms], ins[k][:]) for k in range(nk)]
                    if split:
                        pairs += [(wt[k][:, ms], ins_lo[k][:]) for k in range(nk)]
                        pairs += [(wl[k][:, ms], ins[k][:]) for k in range(nk)]
                    mm_group(p, pairs)
                    o = hpool.tile([128, TB], dt.float16, tag=f"{oname}_{m}")
                    nc.scalar.activation(o[:], p[:], AF.Relu, bias=bt[:, m : m + 1])
                    if drain_deferred and deferred_w:
                        deferred_w.pop(0)(nc.scalar)
                    outs.append(o)
                    if split:
                        of = hpool.tile(
                            [128, TB], dt.float32, tag="hf_tmp", bufs=3
                        )
                        nc.scalar.activation(
                            of[:], p[:], AF.Relu, bias=bt[:, m : m + 1]
                        )
                        ol = hpool.tile([128, TB], dt.float16, tag=f"{oname}l_{m}")
                        nc.vector.tensor_sub(ol[:], of[:], o[:])
                        outs_lo.append(ol)
                return outs, outs_lo

            def l1_tile(bt_i):
                """mT loads + layer 1 for one batch tile (issued one tile
                ahead of layers 2/3 so the PE never stalls on the W2/W3
                arrival at startup, and mT is naturally prefetched)."""
                r0 = bt_i * TB

                def load_mt(dram, tag):
                    big = mpool.tile(
                        [128, 4 * TB], dt.float16, tag=tag, name=tag
                    )
                    nc.sync.dma_start(
                        big[:].rearrange("p (j c) -> p j c", j=4),
                        dram[:, r0 : r0 + TB].rearrange(
                            "(j p) c -> p j c", p=128
                        ),
                    )
                    return [
                        big[:, j * TB : (j + 1) * TB] for j in range(4)
                    ]

                mT = load_mt(mT_d, "mbig")
                mTl = load_mt(mTl_d, "mlbig") if split else []
                return layer(
                    w1t, w1l if split else None, mT, mTl, b1t, H, "h1",
                    drain_deferred=(bt_i == 0),
                )

            h1, h1l = l1_tile(0)
            for bt_i in range(NBT):
                r0 = bt_i * TB

                h1_next = l1_tile(bt_i + 1) if bt_i + 1 < NBT else None

                # x tile (natural layout, needed only for the residual
                # assembly — issued after the mT loads on the same queue).
                # One 3-dim-AP DMA brings all 4 row-chunks side by side.
                xbig = xpool.tile([128, 4 * D], dt.float32, tag="xbig")
                nc.sync.dma_start(
                    xbig[:].rearrange("p (i c) -> p i c", i=4),
                    x_d[r0 : r0 + TB, :].rearrange("(i p) c -> p i c", p=128),
                )
                xb = [xbig[:, i * D : (i + 1) * D] for i in range(4)]
                h2, h2l = layer(
                    w2t, w2l if split else None, h1, h1l, b2t, H, "h2"
                )

                # y is assembled IN PLACE in the x tiles (even columns are
                # already x): odd cols += b3, then += translation.
                for i in range(4):
                    nc.vector.tensor_add(
                        xb[i][:, 1:D:2], xb[i][:, 1:D:2], b3rep[:]
                    )

                # layer 3 in natural layout: stationary = h2 batch-slice,
                # moving = W3 tile  ->  psum[batch128, F]
                for i in range(4):
                    p = pmm.tile([128, F], dt.float32, tag="mm")
                    bs = slice(i * 128, (i + 1) * 128)
                    pairs = [(h2[k][:, bs], w3t[k][:]) for k in range(8)]
                    if split:
                        pairs += [(h2l[k][:, bs], w3t[k][:]) for k in range(8)]
                        pairs += [(h2[k][:, bs], w3l[k][:]) for k in range(8)]
                    mm_group(p, pairs)
                    nc.vector.tensor_add(xb[i][:, 1:D:2], xb[i][:, 1:D:2], p[:])
                    nc.sync.dma_start(
                        y_d[r0 + i * 128 : r0 + (i + 1) * 128, :], xb[i][:]
                    )

                if h1_next is not None:
                    h1, h1l = h1_next

    nc.compile()
    return nc


def _get(mode):
    if mode not in _cache:
        _cache[mode] = _build(mode)
    return _cache[mode]


def _in_maps(x, W1, b1, W2, b2, W3, b3):
    split = MODE == "f16x3"

    def prep_w(w):
        hi = np.asarray(w, dtype=np.float32).astype(np.float16)
        if not split:
            return {"": hi}
        lo = (np.asarray(w, dtype=np.float32) - hi.astype(np.float32)).astype(
            np.float16
        )
        return {"": hi, "l": lo}

    ws = {}
    for name, w in (("w1", W1), ("w2", W2), ("w3", W3)):
        for suf, arr in prep_w(w).items():
            ws[name + suf] = arr

    common = dict(
        ws,
        b1m=np.ascontiguousarray(np.asarray(b1, np.float32).reshape(-1, 128).T),
        b2m=np.ascontiguousarray(np.asarray(b2, np.float32).reshape(-1, 128).T),
        b3rep=np.ascontiguousarray(
            np.broadcast_to(np.asarray(b3, np.float32), (128, F))
        ),
    )
    x = np.ascontiguousarray(np.asarray(x, np.float32))
    in_maps = []
    for c in range(NCORES):
        xs = x[c * BPC : (c + 1) * BPC]
        masked_t = np.ascontiguousarray(xs[:, 0::2].T)  # [F, BPC] f32
        m = dict(common, x=xs, mT=masked_t.astype(np.float16))
        if split:
            m["mTl"] = (masked_t - m["mT"].astype(np.float32)).astype(np.float16)
        in_maps.append(m)
    return in_maps


def kernel(x, W1, b1, W2, b2, W3, b3):
    from concourse.bass_utils import run_bass_kernel_spmd

    nc = _get(MODE)
    res = run_bass_kernel_spmd(
        nc, _in_maps(x, W1, b1, W2, b2, W3, b3), core_ids=list(range(NCORES))
    )
    return np.concatenate([res.results[c]["y"] for c in range(NCORES)], axis=0)


# revision 45
# speedup vs baseline: 2.9295x; 1.1909x over previous
"""Trainium2 Bass kernel for nn_AdditiveCouplingLayer.

y = x; y[:, 1::2] += MLP(x[:, 0::2])  with a 512->1024->1024->512 relu MLP.

Strategy: data-parallel over 8 NeuronCores (batch 65536 -> 8192/core),
weights replicated. The MLP's first two layers run in "transposed
activation" space (features on partitions, batch on the free dim) so
every matmul uses the natural weight layout; the host supplies the
masked half of x pre-transposed and pre-cast to fp16. Layer 3 swaps the
matmul operand roles (h2 slice stationary, W3 moving) so the
translation comes out in natural [batch, feature] layout — no output
transpose needed. Matmuls run in fp16 (1 cycle/row on the PE vs 4 for
fp32) with fp32 PSUM accumulation; weights are pre-cast to fp16 on the
host. MODE "f16x3" upgrades to near-fp32 precision via a 3-term hi/lo
split (3x the matmul work).
"""

import os
import sys

sys.path.insert(0, "/opt/trn_rl_repo")

import numpy as np

B, D, F, H = 65536, 1024, 512, 1024
NCORES = 8
BPC = B // NCORES  # rows per core
TB = 512  # batch tile (matmul free dim)
NBT = BPC // TB  # batch tiles per core
MODE = os.environ.get("BASS_COUPLING_MODE", "f16")

_cache = {}


def _build(mode):
    import concourse.bacc as bacc
    import concourse.tile as tile
    import concourse.mybir as mybir

    dt = mybir.dt
    AF = mybir.ActivationFunctionType
    split = mode == "f16x3"

    nc = bacc.Bacc(
        "TRN2", target_bir_lowering=False, debug=False, num_devices=NCORES
    )

    x_d = nc.dram_tensor("x", [BPC, D], dt.float32, kind="ExternalInput").ap()
    mT_d = nc.dram_tensor("mT", [F, BPC], dt.float16, kind="ExternalInput").ap()
    if split:
        mTl_d = nc.dram_tensor("mTl", [F, BPC], dt.float16, kind="ExternalInput").ap()
    w_d = {}
    for name, shape in (("w1", [F, H]), ("w2", [H, H]), ("w3", [H, F])):
        w_d[name] = nc.dram_tensor(name, shape, dt.float16, kind="ExternalInput").ap()
        if split:
            w_d[name + "l"] = nc.dram_tensor(
                name + "l", shape, dt.float16, kind="ExternalInput"
            ).ap()
    b1_d = nc.dram_tensor("b1m", [128, H // 128], dt.float32, kind="ExternalInput").ap()
    b2_d = nc.dram_tensor("b2m", [128, H // 128], dt.float32, kind="ExternalInput").ap()
    b3r_d = nc.dram_tensor("b3rep", [128, F], dt.float32, kind="ExternalInput").ap()
    y_d = nc.dram_tensor("y", [BPC, D], dt.float32, kind="ExternalOutput").ap()

    with tile.TileContext(nc) as tc:
        with (
            tc.tile_pool(name="wpool", bufs=1) as wpool,
            tc.tile_pool(name="xpool", bufs=2) as xpool,
            tc.tile_pool(name="mpool", bufs=3 if mode == "f16" else 2) as mpool,
            tc.tile_pool(name="hpool", bufs=2) as hpool,
            tc.tile_pool(name="pmm", bufs=4, space="PSUM") as pmm,
        ):
            # --- resident weights/biases ---
            deferred_w = []

            def load_w(name, rows, cols, eng):
                """One big tile + ONE DMA per weight matrix (k-chunks land
                side by side in the free dim); returns per-k column slices.
                eng=None defers the issue (pushed onto deferred_w; drained
                from the scalar engine between the first L1 evictions)."""
                nk = rows // 128
                big = wpool.tile(
                    [128, nk * cols], dt.float16, tag=name, name=name
                )

                def issue(eng):
                    eng.dma_start(
                        big[:].rearrange("p (k c) -> p k c", k=nk),
                        w_d[name].rearrange("(k p) c -> p k c", p=128),
                    )

                if eng is None:
                    deferred_w.append(issue)
                else:
                    issue(eng)
                return [big[:, k * cols : (k + 1) * cols] for k in range(nk)]

            def load_b(name, ap, n):
                # host pre-transposes biases to [128, n/128] so this DMA is
                # contiguous (a "(m p) -> p m" rearrange here is a 4-byte-
                # element gather that takes ~10us and stalls the DMA ring)
                t = wpool.tile([128, n // 128], dt.float32, tag=name)
                nc.scalar.dma_start(t[:], ap[:])
                return t

            # PE warmup: junk matmuls on a zeroed scratch tile keep the PE
            # busy through its HAM activity window while the first real
            # DMAs are in flight, so real matmuls start at 2.4GHz.
            scratch = wpool.tile([128, TB], dt.float16, tag="scratch")
            nc.gpsimd.memset(scratch[:], 0.0)
            pwarm = pmm.tile([128, TB], dt.float32, tag="warm", bufs=1)
            for _ in range(12):
                nc.tensor.matmul(
                    pwarm[:], scratch[:, :128], scratch[:], start=True, stop=True
                )

            # Startup DMA order is the critical path: W1 + tile-0 mT go
            # first on the sync queue; W2/W3 issue from the scalar queue
            # but only AFTER the first L1 evictions (drain_deferred), so
            # they don't steal HBM bandwidth from the W1/mT0 stream.
            # W1 is allocated here but its per-k-chunk DMAs are issued by
            # l1_tile(0) AFTER the tile-0 mT load, so the k-th matmul's
            # operands arrive progressively and the first real matmul can
            # start ~3us earlier than with one monolithic W1 transfer.
            w1big = wpool.tile([128, 4 * H], dt.float16, tag="w1")
            w1t = [w1big[:, k * H : (k + 1) * H] for k in range(4)]
            b1t = load_b("b1t", b1_d, H)
            b2t = load_b("b2t", b2_d, H)
            b3rep = wpool.tile([128, F], dt.float32, tag="b3rep")
            nc.scalar.dma_start(b3rep[:], b3r_d[:])
            w2t = load_w("w2", H, H, None)
            w3t = load_w("w3", H, F, None)
            if split:
                w1lbig = wpool.tile([128, 4 * H], dt.float16, tag="w1l")
                w1l = [w1lbig[:, k * H : (k + 1) * H] for k in range(4)]
                w2l = load_w("w2l", H, H, None)
                w3l = load_w("w3l", H, F, None)

            def mm_group(psum, pairs):
                n = len(pairs)
                for i, (lhsT, rhs) in enumerate(pairs):
                    nc.tensor.matmul(
                        psum[:], lhsT, rhs, start=(i == 0), stop=(i == n - 1)
                    )

            def layer(wt, wl, ins, ins_lo, bt, nout, oname, drain_deferred=False):
                """Transposed-space layer: out[m][feat128, TB] = relu(W.T@in + b)."""
                outs = []
                outs_lo = []
                nk = len(ins)
                for m in range(nout // 128):
                    p = pmm.tile([128, TB], dt.float32, tag="mm")
                    ms = slice(m * 128, (m + 1) * 128)
                    pairs = [(wt[k][:, ms], ins[k][:]) for k in range(nk)]
                    if split:
                        pairs += [(wt[k][:, ms], ins_lo[k][:]) for k in range(nk)]
                        pairs += [(wl[k][:, ms], ins[k][:]) for k in range(nk)]
                    mm_group(p, pairs)
                    o = hpool.tile([128, TB], dt.float16, tag=f"{oname}_{m}")
                    nc.scalar.activation(o[:], p[:], AF.Relu, bias=bt[:, m : m + 1])
                    if drain_deferred and deferred_w:
                        deferred_w.pop(0)(nc.scalar)
                    outs.append(o)
                    if split:
                        of = hpool.tile(
                            [128, TB], dt.float32, tag="hf_tmp", bufs=3
                        )
                        nc.scalar.activation(
                            of[:], p[:], AF.Relu, bias=bt[:, m : m + 1]
                        )
                        ol = hpool.tile([128, TB], dt.float16, tag=f"{oname}l_{m}")
                        nc.vector.tensor_sub(ol[:], of[:], o[:])
                        outs_lo.append(ol)
                return outs, outs_lo

            def l1_tile(bt_i):
                """mT loads + layer 1 for one batch tile (issued one tile
                ahead of layers 2/3 so the PE never stalls on the W2/W3
                arrival at startup, and mT is naturally prefetched)."""
                r0 = bt_i * TB

                def load_mt(dram, tag):
                    big = mpool.tile(
                        [128, 4 * TB], dt.float16, tag=tag, name=tag
                    )
                    nc.sync.dma_start(
                        big[:].rearrange("p (j c) -> p j c", j=4),
                        dram[:, r0 : r0 + TB].rearrange(
                            "(j p) c -> p j c", p=128
                        ),
                    )
                    return [
                        big[:, j * TB : (j + 1) * TB] for j in range(4)
                    ]

                mT = load_mt(mT_d, "mbig")
                mTl = load_mt(mTl_d, "mlbig") if split else []
                if bt_i == 0:
                    for k in range(4):
                        nc.sync.dma_start(
                            w1t[k], w_d["w1"][k * 128 : (k + 1) * 128, :]
                        )
                        if split:
                            nc.sync.dma_start(
                                w1l[k], w_d["w1l"][k * 128 : (k + 1) * 128, :]
                            )
                return layer(
                    w1t, w1l if split else None, mT, mTl, b1t, H, "h1",
                    drain_deferred=(bt_i == 0),
                )

            h1, h1l = l1_tile(0)
            for bt_i in range(NBT):
                r0 = bt_i * TB

                h1_next = l1_tile(bt_i + 1) if bt_i + 1 < NBT else None

                # x tile (natural layout, needed only for the residual
                # assembly — issued after the mT loads on the same queue).
                # One 3-dim-AP DMA brings all 4 row-chunks side by side.
                xbig = xpool.tile([128, 4 * D], dt.float32, tag="xbig")
                nc.sync.dma_start(
                    xbig[:].rearrange("p (i c) -> p i c", i=4),
                    x_d[r0 : r0 + TB, :].rearrange("(i p) c -> p i c", p=128),
                )
                xb = [xbig[:, i * D : (i + 1) * D] for i in range(4)]
                h2, h2l = layer(
                    w2t, w2l if split else None, h1, h1l, b2t, H, "h2"
                )

                # y is assembled IN PLACE in the x tiles (even columns are
                # already x): odd cols += b3, then += translation.
                for i in range(4):
                    nc.vector.tensor_add(
                        xb[i][:, 1:D:2], xb[i][:, 1:D:2], b3rep[:]
                    )

                # layer 3 in natural layout: stationary = h2 batch-slice,
                # moving = W3 tile  ->  psum[batch128, F]
                for i in range(4):
                    p = pmm.tile([128, F], dt.float32, tag="mm")
                    bs = slice(i * 128, (i + 1) * 128)
                    pairs = [(h2[k][:, bs], w3t[k][:]) for k in range(8)]
                    if split:
                        pairs += [(h2l[k][:, bs], w3t[k][:]) for k in range(8)]
                        pairs += [(h2[k][:, bs], w3l[k][:]) for k in range(8)]
                    mm_group(p, pairs)
                    rows = y_d[r0 + i * 128 : r0 + (i + 1) * 128, :]
                    if bt_i == NBT - 1:
                        # final tile: halve the add+store chain so the
                        # kernel tail after the last matmul is shorter
                        for h in range(2):
                            osl = slice(h * F + 1, (h + 1) * F, 2)
                            nc.vector.tensor_add(
                                xb[i][:, osl], xb[i][:, osl],
                                p[:, h * (F // 2) : (h + 1) * (F // 2)],
                            )
                            nc.sync.dma_start(
                                rows[:, h * F : (h + 1) * F],
                                xb[i][:, h * F : (h + 1) * F],
                            )
                    else:
                        nc.vector.tensor_add(
                            xb[i][:, 1:D:2], xb[i][:, 1:D:2], p[:]
                        )
                        nc.sync.dma_start(rows[:], xb[i][:])

                if h1_next is not None:
                    h1, h1l = h1_next

    nc.compile()
    return nc


def _get(mode):
    if mode not in _cache:
        _cache[mode] = _build(mode)
    return _cache[mode]


def _in_maps(x, W1, b1, W2, b2, W3, b3):
    split = MODE == "f16x3"

    def prep_w(w):
        hi = np.asarray(w, dtype=np.float32).astype(np.float16)
        if not split:
            return {"": hi}
        lo = (np.asarray(w, dtype=np.float32) - hi.astype(np.float32)).astype(
            np.float16
        )
        return {"": hi, "l": lo}

    ws = {}
    for name, w in (("w1", W1), ("w2", W2), ("w3", W3)):
        for suf, arr in prep_w(w).items():
            ws[name + suf] = arr

    common = dict(
        ws,
        b1m=np.ascontiguousarray(np.asarray(b1, np.float32).reshape(-1, 128).T),
        b2m=np.ascontiguousarray(np.asarray(b2, np.float32).reshape(-1, 128).T),
        b3rep=np.ascontiguousarray(
            np.broadcast_to(np.asarray(b3, np.float32), (128, F))
        ),
    )
    x = np.ascontiguousarray(np.asarray(x, np.float32))
    in_maps = []
    for c in range(NCORES):
        xs = x[c * BPC : (c + 1) * BPC]
        masked_t = np.ascontiguousarray(xs[:, 0::2].T)  # [F, BPC] f32
        m = dict(common, x=xs, mT=masked_t.astype(np.float16))
        if split:
            m["mTl"] = (masked_t - m["mT"].astype(np.float32)).astype(np.float16)
        in_maps.append(m)
    return in_maps


def kernel(x, W1, b1, W2, b2, W3, b3):
    from concourse.bass_utils import run_bass_kernel_spmd

    nc = _get(MODE)
    res = run_bass_kernel_spmd(
        nc, _in_maps(x, W1, b1, W2, b2, W3, b3), core_ids=list(range(NCORES))
    )
    return np.concatenate([res.results[c]["y"] for c in range(NCORES)], axis=0)
